# revision 1
# baseline (speedup 1.0000x reference)
"""Mamba (4-layer) Trainium2 Bass kernel.

Sharding: 8 cores = E-quarter(4) x batch-pair(2).  Core c handles channel
quarter q = c%4 of d_inner for TWO batches (0,1 if c<4 else 2,3).  Quad
replica groups [[0..3],[4..7]] AllReduce (a) the x_proj partial [80,L] and
(b) the out_proj partial [768,L] per batch per layer.  The two batch
streams per core are interleaved so one batch's compute hides the other
batch's AllReduce latency.

Engine split: PE matmuls; DVE scans + small elementwise; Pool (gpsimd)
does the N*L-domain elementwise (dBu, y*C, n-tree reduction) + PSUM
copies; ACT does exp/silu/softplus; B/C partition-broadcast is a single
stride-0 DMA from the AllReduce DRAM buffer.
"""

import sys

sys.path.insert(0, "/opt/trn_rl_repo")

import numpy as np
import ml_dtypes

bf16 = ml_dtypes.bfloat16

# model dims (hardcoded from the problem spec)
B, L, IN_DIM, OUT_DIM = 4, 512, 32, 1
D, NL = 768, 4
E = 2 * D          # 1536
EQ = E // 4        # 384 channels per core (quarter)
ETQ = EQ // 128    # 3 e-tiles per unit
N = 16
K = 4
R = D // 16        # 48
NC = 8             # cores
DK = D // 128      # 6 d-tiles
XD = R + 2 * N     # 80

_BUILT = {}


def _legalize_waits(nc, mybir, max_waits=1):
    """This walrus build rejects >1 sem-wait per instruction: hoist extras
    onto preceding same-engine NoOps (streams execute in order)."""
    ctr = 0
    for fn in nc.m.functions:
        for bb in fn.blocks:
            insts = bb.instructions
            out = []
            dirty = False
            for inst in insts:
                si = inst.sync_info
                if si is not None and len(si.on_wait) > max_waits:
                    waits = list(si.on_wait)
                    extra, keep = waits[:-max_waits], waits[-max_waits:]
                    for i in range(0, len(extra), max_waits):
                        ctr += 1
                        nop = mybir.InstNoOp(name=f"I-waitfix-{ctr}", ins=[], outs=[])
                        nop.engine = inst.engine
                        nop.sync_info = mybir.SyncInfo(
                            on_wait=extra[i : i + max_waits], on_update=[]
                        )
                        out.append(nop)
                    inst.sync_info = mybir.SyncInfo(
                        on_wait=keep, on_update=list(si.on_update)
                    )
                    dirty = True
                out.append(inst)
            if dirty:
                bb.instructions = out


def _build():
    if "nc" in _BUILT:
        return _BUILT["nc"]

    import concourse.bass as bass
    import concourse.tile as tile
    from concourse import mybir

    FP32 = mybir.dt.float32
    BF16 = mybir.dt.bfloat16
    AF = mybir.ActivationFunctionType
    OP = mybir.AluOpType

    nc = bass.Bass("TRN2", target_bir_lowering=False, debug=False, num_devices=NC)

    # ---- dram I/O ----
    dt_in = lambda name, shape, dt: nc.dram_tensor(name, shape, dt, kind="ExternalInput")
    xt = dt_in("xt", [2, IN_DIM, L], BF16)           # two batches, transposed
    w_in = dt_in("w_in", [IN_DIM, D], BF16)          # in_w.T
    b_in = dt_in("b_in", [D, 1], FP32)
    wxc = dt_in("wxc", [NL, D, EQ], BF16)            # (in_proj xc-quarter * norm_w).T
    wres = dt_in("wres", [NL, D, EQ], BF16)          # (in_proj res-quarter * norm_w).T
    wxp = dt_in("wxp", [NL, EQ, XD], BF16)           # xproj quarter .T
    wdt = dt_in("wdt", [NL, R, EQ], BF16)            # dtproj quarter .T
    bdt = dt_in("bdt", [NL, EQ, 1], FP32)
    wcv = dt_in("wcv", [NL, EQ, K], FP32)
    bcv = dt_in("bcv", [NL, EQ, 1], FP32)
    a_neg = dt_in("a_neg", [NL, EQ, N], FP32)        # -exp(A_log) quarter
    dssm = dt_in("dssm", [NL, EQ, 1], FP32)
    wo = dt_in("wo", [NL, EQ, D], BF16)              # outproj quarter .T  [e, d]
    who = dt_in("who", [D, 1], BF16)                 # (out_w * normf_w).T
    ob = dt_in("ob", [1, 1], FP32)
    out_t = nc.dram_tensor("out", [2, 1], FP32, kind="ExternalOutput")

    groups = [[0, 1, 2, 3], [4, 5, 6, 7]]

    with tile.TileContext(nc) as tc:
        import contextlib

        ctx = contextlib.ExitStack()
        with ctx:
            pool = ctx.enter_context(tc.tile_pool(name="main", bufs=1))
            hpool = ctx.enter_context(tc.tile_pool(name="h", bufs=13))
            wpool = ctx.enter_context(tc.tile_pool(name="w", bufs=1))
            w1pool = ctx.enter_context(tc.tile_pool(name="w1", bufs=1))
            xnpool = ctx.enter_context(tc.tile_pool(name="xn", bufs=7))
            apool = ctx.enter_context(tc.tile_pool(name="acts", bufs=6))
            a4pool = ctx.enter_context(tc.tile_pool(name="acts4", bufs=4))
            scpool = ctx.enter_context(tc.tile_pool(name="scan", bufs=4))
            bcpool = ctx.enter_context(tc.tile_pool(name="bc", bufs=1))
            smpool = ctx.enter_context(tc.tile_pool(name="small", bufs=2))
            tinyp = ctx.enter_context(tc.tile_pool(name="tiny", bufs=12))
            sm1 = ctx.enter_context(tc.tile_pool(name="sm1", bufs=1))
            pspool = ctx.enter_context(tc.tile_pool(name="ps", bufs=4, space="PSUM"))
            psb = ctx.enter_context(tc.tile_pool(name="psb", bufs=1, space="PSUM"))
            pss = ctx.enter_context(tc.tile_pool(name="pss", bufs=1, space="PSUM"))
            dram = ctx.enter_context(tc.tile_pool(name="dram", bufs=2, space="DRAM"))

            # constants
            ones_c = pool.tile([128, 1], BF16)   # column of ones (sumsq lhsT)
            nc.vector.memset(ones_c[:], 1.0)
            ones_r = pool.tile([1, 128], FP32)   # row of ones (bcast lhsT)
            nc.vector.memset(ones_r[:], 1.0)
            who_sb = pool.tile([128, DK], BF16)
            nc.sync.dma_start(who_sb[:].unsqueeze(2), who.ap().rearrange("(k p) o -> p k o", p=128))
            ob_sb = pool.tile([1, 1], FP32)
            nc.sync.dma_start(ob_sb[:], ob.ap())
            eps_sb = pool.tile([1, 1], FP32)
            nc.vector.memset(eps_sb[:], 1e-5)
            onec_f = pool.tile([128, 1], FP32)
            nc.vector.memset(onec_f[:], 1.0)
            ones_L = pool.tile([128, L], BF16)
            nc.vector.memset(ones_L[:], 1.0)

            # ---- input projection: h0 = in_w @ x + b (per unit) ----
            win_sb = pool.tile([IN_DIM, D], BF16)
            nc.sync.dma_start(win_sb[:], w_in.ap())
            bin_sb = pool.tile([128, DK], FP32)
            nc.sync.dma_start(bin_sb[:].unsqueeze(2), b_in.ap().rearrange("(k p) o -> p k o", p=128))

            # warmup barrier: absorb first-collective sync skew behind the
            # input DMAs / first matmuls
            wrm_i = dram.tile([1, 1], BF16, tag="wrmi")
            wrm_o = dram.tile([1, 1], BF16, tag="wrmo")
            nc.sync.dma_start(wrm_i[:], ones_c[0:1, 0:1])
            nc.gpsimd.collective_compute(
                "AllReduce", OP.add, replica_groups=groups,
                ins=[wrm_i.opt()], outs=[wrm_o.opt()])

            hres = [[], []]
            for u in range(2):
                xt_sb = sm1.tile([IN_DIM, L], BF16, tag="xt")
                nc.sync.dma_start(xt_sb[:], xt.ap()[u])
                for k in range(DK):
                    ps = pspool.tile([128, L], FP32)
                    nc.tensor.matmul(ps[:], win_sb[:, k * 128 : (k + 1) * 128], xt_sb[:],
                                     start=True, stop=True)
                    hk = hpool.tile([128, L], BF16, tag="hres")
                    nc.scalar.activation(hk[:], ps[:], AF.Identity, bias=bin_sb[:, k : k + 1])
                    hres[u].append(hk)

            # per-unit comm buffers (dram pool bufs=2 rotates across layers)
            def comm(u):
                a1i = dram.tile([XD, L], BF16, tag=f"a1i{u}")
                a1o = dram.tile([XD, L], BF16, tag=f"a1o{u}")
                a2i = dram.tile([D, L], BF16, tag=f"a2i{u}")
                a2o = dram.tile([D, L], BF16, tag=f"a2o{u}")
                return a1i, a1o, a2i, a2o

            prev_ar2 = [None, None]  # residual carried via AR2 out of prev layer

            for l in range(NL):
                # ---- layer weights (double-buffered; shared by both units) ----
                wxc_sb = wpool.tile([128, DK * EQ], BF16, tag="wxc")
                nc.sync.dma_start(wxc_sb[:].rearrange("p (k e) -> p k e", k=DK), wxc.ap()[l].rearrange("(k p) e -> p k e", p=128))
                wres_sb = wpool.tile([128, DK * EQ], BF16, tag="wres")
                nc.sync.dma_start(wres_sb[:].rearrange("p (k e) -> p k e", k=DK), wres.ap()[l].rearrange("(k p) e -> p k e", p=128))
                wo_sb = wpool.tile([128, ETQ * D], BF16, tag="wo")
                nc.sync.dma_start(wo_sb[:].rearrange("p (k d) -> p k d", k=ETQ), wo.ap()[l].rearrange("(k p) d -> p k d", p=128))
                wxp_sb = w1pool.tile([128, ETQ * XD], BF16, tag="wxp")
                nc.sync.dma_start(wxp_sb[:].rearrange("p (k r) -> p k r", k=ETQ), wxp.ap()[l].rearrange("(k p) r -> p k r", p=128))
                wdt_sb = w1pool.tile([R, EQ], BF16, tag="wdt")
                nc.sync.dma_start(wdt_sb[:], wdt.ap()[l])
                bdt_sb = w1pool.tile([128, ETQ], FP32, tag="bdt")
                nc.sync.dma_start(bdt_sb[:].unsqueeze(2), bdt.ap()[l].rearrange("(k p) o -> p k o", p=128))
                wcv_sb = w1pool.tile([128, ETQ * K], FP32, tag="wcv")
                nc.sync.dma_start(wcv_sb[:].rearrange("p (k c) -> p k c", k=ETQ), wcv.ap()[l].rearrange("(k p) c -> p k c", p=128))
                bcv_sb = w1pool.tile([128, ETQ], FP32, tag="bcv")
                nc.sync.dma_start(bcv_sb[:].unsqueeze(2), bcv.ap()[l].rearrange("(k p) o -> p k o", p=128))
                a_sb = w1pool.tile([128, ETQ * N], FP32, tag="a")
                nc.sync.dma_start(a_sb[:].rearrange("p (k n) -> p k n", k=ETQ), a_neg.ap()[l].rearrange("(k p) n -> p k n", p=128))
                dssm_sb = w1pool.tile([128, ETQ], FP32, tag="dssm")
                nc.sync.dma_start(dssm_sb[:].unsqueeze(2), dssm.ap()[l].rearrange("(k p) o -> p k o", p=128))

                C = [comm(0), comm(1)]
                U = [{}, {}]  # per-unit state

                # ================= P1a: residual + rmsnorm + xn (both units)
                for u in range(2):
                    st = U[u]
                    if prev_ar2[u] is not None:
                        hnew = []
                        for k in range(DK):
                            pr = smpool.tile([128, L], BF16, tag="pp")
                            nc.sync.dma_start(pr[:], prev_ar2[u][k * 128 : (k + 1) * 128, :])
                            hk = hpool.tile([128, L], BF16, tag="hres")
                            nc.vector.tensor_add(hk[:], hres[u][k][:], pr[:])
                            hnew.append(hk)
                        hres[u] = hnew

                    # rmsnorm rstd (norm_w folded into weights)
                    ssq = pss.tile([1, L], FP32, tag="ssq")
                    for k in range(DK):
                        hsq = smpool.tile([128, L], BF16, tag="hsq")
                        nc.scalar.activation(hsq[:], hres[u][k][:], AF.Square)
                        nc.tensor.matmul(ssq[:], ones_c[:], hsq[:],
                                         start=(k == 0), stop=(k == DK - 1))
                    lnms = pss.tile([1, L], FP32, tag="std")
                    nc.scalar.activation(lnms[:], ssq[:], AF.Ln, scale=1.0 / D, bias=eps_sb[:])
                    rstd = sm1.tile([1, L], FP32, tag="rstd")
                    nc.scalar.activation(rstd[:], lnms[:], AF.Exp, scale=-0.5)
                    rstd_bc = pss.tile([128, L], FP32, tag="rstdbc")
                    nc.tensor.matmul(rstd_bc[:], ones_r[:], rstd[:], start=True, stop=True)
                    rstd_sb = sm1.tile([128, L], BF16, tag="rstdsb")
                    nc.scalar.activation(rstd_sb[:], rstd_bc[:], AF.Copy)
                    xn = []
                    for k in range(DK):
                        xnk = xnpool.tile([128, L], BF16, tag="xn")
                        nc.vector.tensor_mul(xnk[:], hres[u][k][:], rstd_sb[:])
                        xn.append(xnk)

                    # in_proj -> xc (conv input, padded); res -> gate
                    xc = []
                    for ek in range(ETQ):
                        ps = pspool.tile([128, L], FP32)
                        for dk in range(DK):
                            nc.tensor.matmul(
                                ps[:],
                                wxc_sb[:, dk * EQ + ek * 128 : dk * EQ + (ek + 1) * 128],
                                xn[dk][:], start=(dk == 0), stop=(dk == DK - 1))
                        xck = a4pool.tile([128, L + K - 1], BF16, tag="xc")
                        nc.vector.memset(xck[:, 0 : K - 1], 0.0)
                        nc.scalar.activation(xck[:, K - 1 :], ps[:], AF.Copy)
                        xc.append(xck)
                    g = []
                    GL = 1 if l == NL - 1 else L  # last layer: gate only at t=L-1
                    for ek in range(ETQ):
                        ps = pspool.tile([128, GL], FP32)
                        for dk in range(DK):
                            nc.tensor.matmul(
                                ps[:],
                                wres_sb[:, dk * EQ + ek * 128 : dk * EQ + (ek + 1) * 128],
                                xn[dk][:, L - GL :], start=(dk == 0), stop=(dk == DK - 1))
                        gk = apool.tile([128, GL], BF16, tag="g")
                        nc.scalar.activation(gk[:], ps[:], AF.Silu)
                        g.append(gk)
                    st["g"] = g

                    # depthwise causal conv (1 ts_mul + 3 fused stt) + silu -> u
                    ut = []
                    for ek in range(ETQ):
                        cm = []
                        for kk in range(K):
                            ck = smpool.tile([128, L], BF16, tag=f"cva{kk % 2}")
                            nc.vector.tensor_scalar_mul(
                                ck[:], xc[ek][:, kk : kk + L],
                                wcv_sb[:, ek * K + kk : ek * K + kk + 1])
                            cm.append(ck)
                        c01 = smpool.tile([128, L], BF16, tag="cvb0")
                        nc.vector.tensor_add(c01[:], cm[0][:], cm[1][:])
                        c23 = smpool.tile([128, L], BF16, tag="cvb1")
                        nc.vector.tensor_add(c23[:], cm[2][:], cm[3][:])
                        ca = smpool.tile([128, L], BF16, tag="cvc")
                        nc.vector.tensor_add(ca[:], c01[:], c23[:])
                        uk = apool.tile([128, L], BF16, tag="u")
                        nc.scalar.activation(uk[:], ca[:], AF.Silu, bias=bcv_sb[:, ek : ek + 1])
                        ut.append(uk)
                    st["u"] = ut

                    # x_proj partial -> AR1 fire
                    ps_xd = psb.tile([XD, L], FP32, tag="psxd")
                    for ek in range(ETQ):
                        nc.tensor.matmul(
                            ps_xd[:],
                            wxp_sb[:, ek * XD : (ek + 1) * XD],
                            ut[ek][:], start=(ek == 0), stop=(ek == ETQ - 1))
                    xdp = sm1.tile([XD, L], BF16, tag="xdp")
                    nc.scalar.activation(xdp[:], ps_xd[:], AF.Copy)
                    a1i, a1o, _, _ = C[u]
                    nc.sync.dma_start(a1i[:], xdp[:])
                    nc.gpsimd.collective_compute(
                        "AllReduce", OP.add, replica_groups=groups,
                        ins=[a1i.opt()], outs=[a1o.opt()])

                # ================= P2 (+ inline out_proj/AR2 per unit)
                for u in range(2):
                    st = U[u]
                    a1i, a1o, _, _ = C[u]
                    ut, g = st["u"], st["g"]

                    dt_bf = sm1.tile([R, L], BF16, tag="dtbf")
                    nc.sync.dma_start(dt_bf[:], a1o[0:R, :])
                    # B/C rows broadcast to all 128 partitions.  Issued on the
                    # ACT DMA queue so the small SP-queue DMAs don't stall
                    # behind these 2 MB streams; C lands later than B (it is
                    # consumed ~a scan later), and the last layer only needs
                    # B plus the final C column.
                    bc_sb = bcpool.tile([128, 2 * N * L], BF16, tag="bc")
                    nc.scalar.dma_start(
                        bc_sb[:, 0 : N * L],
                        a1o[R : R + N, :].flatten().partition_broadcast(128),
                    )
                    Bbc = bc_sb[:, 0 : N * L]
                    Cbc = bc_sb[:, N * L : 2 * N * L]
                    if l < NL - 1:
                        nc.scalar.dma_start(
                            Cbc,
                            a1o[R + N : R + 2 * N, :].flatten().partition_broadcast(128),
                        )
                    else:
                        Ccol = smpool.tile([128, N], BF16, tag="ccol")
                        nc.scalar.dma_start(
                            Ccol[:],
                            a1o[R + N : R + 2 * N, L - 1 : L].squeeze().partition_broadcast(128),
                        )

                    yg = []

                    def delta_du(ek):
                        # delta = softplus(dtproj @ dt + bias); du = delta * u
                        ps = pspool.tile([128, L], FP32)
                        nc.tensor.matmul(ps[:], wdt_sb[:, ek * 128 : (ek + 1) * 128],
                                         dt_bf[:], start=True, stop=True)
                        zabs = smpool.tile([128, L], BF16, tag="spa")
                        nc.scalar.activation(zabs[:], ps[:], AF.Abs,
                                             bias=bdt_sb[:, ek : ek + 1])
                        zrelu = smpool.tile([128, L], BF16, tag="spr")
                        nc.scalar.activation(zrelu[:], ps[:], AF.Relu,
                                             bias=bdt_sb[:, ek : ek + 1])
                        esp = smpool.tile([128, L], BF16, tag="spa")
                        nc.scalar.activation(esp[:], zabs[:], AF.Exp, scale=-1.0)
                        ln1p = smpool.tile([128, L], BF16, tag="spa")
                        nc.scalar.activation(ln1p[:], esp[:], AF.Ln, bias=onec_f[:])
                        dk_t = a4pool.tile([128, L], BF16, tag="delta")
                        nc.vector.tensor_add(dk_t[:], zrelu[:], ln1p[:])
                        du = a4pool.tile([128, L], BF16, tag="du")
                        nc.vector.tensor_mul(du[:], dk_t[:], ut[ek][:])
                        return dk_t, du

                    if l == NL - 1:
                        # last layer: only the final state h[:, :, L-1] is
                        # needed.  h_end[n] = sum_s exp(A_n (S_end - S_s)) dBu_s
                        # with S = cumsum(delta); exponent is always <= 0.
                        # Queue all exps first so ACT streams ahead of the DVE sweeps.
                        Ps, dus = [], []
                        for ek in range(ETQ):
                            dk_t, du = delta_du(ek)
                            S = smpool.tile([128, L], FP32, tag="cs")
                            nc.vector.tensor_tensor_scan(
                                S[:], ones_L[:], dk_t[:], 0.0, OP.mult, OP.add)
                            T = smpool.tile([128, L], FP32, tag="ctl")
                            nc.vector.tensor_sub(
                                T[:], S[:, L - 1 : L].broadcast_to([128, L]), S[:])
                            P = scpool.tile([128, N * L], BF16, tag="sc")
                            for n in range(N):
                                nc.scalar.activation(
                                    P[:, n * L : (n + 1) * L], T[:], AF.Exp,
                                    scale=a_sb[:, ek * N + n : ek * N + n + 1])
                            Ps.append(P)
                            dus.append(du)
                        for ek in range(ETQ):
                            P, du = Ps[ek], dus[ek]
                            dBu = scpool.tile([128, N * L], BF16, tag="sc")
                            nc.vector.tensor_mul(
                                dBu[:].rearrange("p (n t) -> p n t", n=N),
                                du[:].unsqueeze(1).broadcast_to([128, N, L]),
                                Bbc.rearrange("p (n t) -> p n t", n=N))
                            nc.vector.tensor_mul(P[:], P[:], dBu[:])
                            h511 = smpool.tile([128, N], FP32, tag="h511")
                            nc.vector.tensor_reduce(
                                h511[:], P[:].rearrange("p (n t) -> p n t", n=N),
                                mybir.AxisListType.X, OP.add)
                            yCl = smpool.tile([128, N], BF16, tag="yCl")
                            nc.vector.tensor_mul(yCl[:], h511[:], Ccol[:])
                            ysum = smpool.tile([128, 1], FP32, tag="ysum")
                            nc.vector.tensor_reduce(ysum[:], yCl[:], mybir.AxisListType.X, OP.add)
                            ud = smpool.tile([128, 1], BF16, tag="ud")
                            nc.vector.tensor_scalar_mul(ud[:], ut[ek][:, L - 1 : L],
                                                        dssm_sb[:, ek : ek + 1])
                            yd = smpool.tile([128, 1], BF16, tag="yd")
                            nc.vector.tensor_add(yd[:], ud[:], ysum[:])
                            ygk = a4pool.tile([128, 1], BF16, tag="ygk")
                            nc.vector.tensor_mul(ygk[:], yd[:], g[ek][:])
                            yg.append(ygk)
                        st["yg"] = yg
                        continue

                    for ek in range(ETQ):
                        dk_t, du = delta_du(ek)
                        dA = scpool.tile([128, N * L], BF16, tag="sc")
                        for n in range(N):
                            nc.scalar.activation(
                                dA[:, n * L : (n + 1) * L], dk_t[:], AF.Exp,
                                scale=a_sb[:, ek * N + n : ek * N + n + 1])
                        # zero the t=0 column of every n-chain (multiplies the
                        # zero initial state) so one scan spans all 16 chains
                        nc.vector.memset(dA[:].rearrange("p (n t) -> p n t", n=N)[:, :, 0:1], 0.0)
                        dBu = scpool.tile([128, N * L], BF16, tag="sc")
                        nc.vector.tensor_mul(
                            dBu[:].rearrange("p (n t) -> p n t", n=N),
                            du[:].unsqueeze(1).broadcast_to([128, N, L]),
                            Bbc.rearrange("p (n t) -> p n t", n=N))
                        hsc = scpool.tile([128, N * L], BF16, tag="sc")
                        nc.vector.tensor_tensor_scan(
                            hsc[:], dA[:], dBu[:], 0.0, OP.mult, OP.add)
                        # y*C and the n-tree reduction run in place in hsc
                        nc.vector.tensor_mul(hsc[:], hsc[:], Cbc)
                        v = hsc[:].rearrange("p (n t) -> p n t", n=N)
                        nc.vector.tensor_add(
                            v[:, 0 : N // 2, :], v[:, 0 : N // 2, :], v[:, N // 2 : N, :])
                        nc.vector.tensor_add(
                            v[:, 0 : N // 4, :], v[:, 0 : N // 4, :], v[:, N // 4 : N // 2, :])
                        nc.vector.tensor_add(
                            v[:, 0 : N // 8, :], v[:, 0 : N // 8, :], v[:, N // 8 : N // 4, :])
                        yssm = smpool.tile([128, L], BF16, tag="yssm")
                        nc.vector.tensor_add(yssm[:], hsc[:, 0:L], hsc[:, L : 2 * L])
                        ud = smpool.tile([128, L], BF16, tag="ud")
                        nc.vector.tensor_scalar_mul(ud[:], ut[ek][:], dssm_sb[:, ek : ek + 1])
                        yd = smpool.tile([128, L], BF16, tag="yd")
                        nc.vector.tensor_add(yd[:], ud[:], yssm[:])
                        ygk = a4pool.tile([128, L], BF16, tag="ygk")
                        nc.vector.tensor_mul(ygk[:], yd[:], g[ek][:])
                        yg.append(ygk)
                    st["yg"] = yg

                    if l < NL - 1:
                        # fire this unit's out_proj AllReduce immediately: it
                        # flies while the other unit's P2 computes
                        _, _, a2i, a2o = C[u]
                        for dk in range(DK):
                            ps = pspool.tile([128, L], FP32)
                            for ek in range(ETQ):
                                nc.tensor.matmul(
                                    ps[:],
                                    wo_sb[:, ek * D + dk * 128 : ek * D + (dk + 1) * 128],
                                    yg[ek][:], start=(ek == 0), stop=(ek == ETQ - 1))
                            pf = smpool.tile([128, L], BF16, tag="pp")
                            nc.scalar.activation(pf[:], ps[:], AF.Copy)
                            nc.sync.dma_start(a2i[dk * 128 : (dk + 1) * 128, :], pf[:])
                        nc.gpsimd.collective_compute(
                            "AllReduce", OP.add, replica_groups=groups,
                            ins=[a2i.opt()], outs=[a2o.opt()])
                        prev_ar2[u] = a2o

                # ================= P3: last layer only — merged tiny AR2
                if l == NL - 1:
                    a2i = dram.tile([2 * D, 1], BF16, tag="a2is")
                    a2o = dram.tile([2 * D, 1], BF16, tag="a2os")
                    for u in range(2):
                        st = U[u]
                        for dk in range(DK):
                            ps = pspool.tile([128, 1], FP32)
                            for ek in range(ETQ):
                                nc.tensor.matmul(
                                    ps[:],
                                    wo_sb[:, ek * D + dk * 128 : ek * D + (dk + 1) * 128],
                                    st["yg"][ek][:], start=(ek == 0), stop=(ek == ETQ - 1))
                            pf = tinyp.tile([128, 1], BF16, tag="ppl")
                            nc.scalar.activation(pf[:], ps[:], AF.Copy)
                            nc.scalar.dma_start(
                                a2i[u * D + dk * 128 : u * D + (dk + 1) * 128, :], pf[:])
                    nc.gpsimd.collective_compute(
                        "AllReduce", OP.add, replica_groups=groups,
                        ins=[a2i.opt()], outs=[a2o.opt()])
                    prev_ar2 = [a2o, a2o]
                    ar2_off = [0, D]


            # ---- final: residual + rmsnorm(last token) + head + sigmoid ----
            for u in range(2):
                hnew = []
                for k in range(DK):
                    pr = tinyp.tile([128, 1], BF16, tag="prl")
                    nc.scalar.dma_start(
                        pr[:], prev_ar2[u][ar2_off[u] + k * 128 : ar2_off[u] + (k + 1) * 128, :])
                    hk = hpool.tile([128, 1], BF16, tag="hlast")
                    nc.vector.tensor_add(hk[:], hres[u][k][:, L - 1 : L], pr[:])
                    hnew.append(hk)
                hres[u] = hnew

                ssq2 = pss.tile([1, 1], FP32, tag="ssq")
                dot = pss.tile([1, 1], FP32, tag="rstdbc")
                for k in range(DK):
                    hl_bf = hres[u][k]
                    sq = smpool.tile([128, 1], BF16, tag="hlsq")
                    nc.scalar.activation(sq[:], hres[u][k][:], AF.Square)
                    nc.tensor.matmul(ssq2[:], ones_c[:], sq[:],
                                     start=(k == 0), stop=(k == DK - 1))
                    nc.tensor.matmul(dot[:], hl_bf[:], who_sb[:, k : k + 1],
                                     start=(k == 0), stop=(k == DK - 1))
                lnms2 = smpool.tile([1, 1], FP32, tag="std2")
                nc.scalar.activation(lnms2[:], ssq2[:], AF.Ln, scale=1.0 / D, bias=eps_sb[:])
                rstd2 = smpool.tile([1, 1], FP32, tag="rstd2")
                nc.scalar.activation(rstd2[:], lnms2[:], AF.Exp, scale=-0.5)
                logit = smpool.tile([1, 1], FP32, tag="logit")
                nc.vector.tensor_mul(logit[:], dot[:], rstd2[:])
                res = smpool.tile([1, 1], FP32, tag="res")
                nc.scalar.activation(res[:], logit[:], AF.Sigmoid, bias=ob_sb[:])
                nc.sync.dma_start(out_t.ap()[u : u + 1, :], res[:])

    _legalize_waits(nc, mybir)
    _BUILT["nc"] = nc
    return nc


def _pack_inputs(inputs):
    """Per-core input dicts from the full-model inputs."""
    f32 = lambda a: np.asarray(a, np.float32)
    x = f32(inputs["x"])                    # [B, L, 32]
    in_w = f32(inputs["in_w"])              # [D, 32]
    in_b = f32(inputs["in_b"])              # [D]
    in_proj_w = f32(inputs["in_proj_w"])    # [NL, 2E, D]
    conv_w = f32(inputs["conv_w"])          # [NL, E, K]
    conv_b = f32(inputs["conv_b"])          # [NL, E]
    xproj_w = f32(inputs["xproj_w"])        # [NL, R+2N, E]
    dtproj_w = f32(inputs["dtproj_w"])      # [NL, E, R]
    dtproj_b = f32(inputs["dtproj_b"])      # [NL, E]
    A_log = f32(inputs["A_log"])            # [NL, E, N]
    D_ssm = f32(inputs["D_ssm"])            # [NL, E]
    outproj_w = f32(inputs["outproj_w"])    # [NL, D, E]
    norm_w = f32(inputs["norm_w"])          # [NL, D]
    normf_w = f32(inputs["normf_w"])        # [D]
    out_w = f32(inputs["out_w"])            # [1, D]
    out_b = f32(inputs["out_b"])            # [1]

    per_q = []
    for q in range(4):
        sl = slice(q * EQ, (q + 1) * EQ)
        wxc_q = np.stack([
            (in_proj_w[l, sl, :] * norm_w[l][None, :]).T for l in range(NL)])
        wres_q = np.stack([
            (in_proj_w[l, E + q * EQ : E + (q + 1) * EQ, :] * norm_w[l][None, :]).T
            for l in range(NL)])
        wxp_q = np.stack([xproj_w[l][:, sl].T for l in range(NL)])
        wdt_q = np.stack([dtproj_w[l, sl, :].T for l in range(NL)])
        wo_q = np.stack([outproj_w[l][:, sl].T for l in range(NL)])
        per_q.append(dict(
            wxc=wxc_q.astype(bf16), wres=wres_q.astype(bf16),
            wxp=wxp_q.astype(bf16), wdt=wdt_q.astype(bf16),
            wo=wo_q.astype(bf16),
            bdt=dtproj_b[:, sl, None].astype(np.float32),
            wcv=conv_w[:, sl, :].astype(np.float32),
            bcv=conv_b[:, sl, None].astype(np.float32),
            a_neg=(-np.exp(A_log[:, sl, :])).astype(np.float32),
            dssm=D_ssm[:, sl, None].astype(np.float32),
        ))

    shared = dict(
        w_in=in_w.T.astype(bf16),
        b_in=in_b[:, None].astype(np.float32),
        who=(out_w[0] * normf_w)[:, None].astype(bf16),
        ob=np.array([[out_b[0]]], np.float32),
    )

    in_maps = []
    for c in range(NC):
        q, bp = c % 4, c // 4
        m = dict(shared)
        m["xt"] = np.stack([x[2 * bp].T, x[2 * bp + 1].T]).astype(bf16)
        m.update(per_q[q])
        in_maps.append(m)
    return in_maps


def kernel(**inputs) -> np.ndarray:
    from concourse.bass_utils import run_bass_kernel_spmd

    nc = _build()
    in_maps = _pack_inputs(inputs)
    res = run_bass_kernel_spmd(nc, in_maps, core_ids=list(range(NC)))
    out = np.array(
        [res.results[0]["out"][0, 0], res.results[0]["out"][1, 0],
         res.results[4]["out"][0, 0], res.results[4]["out"][1, 0]],
        np.float32,
    )
    return out


if __name__ == "__main__":
    rng = np.random.default_rng(0)
    ins = {
        "x": rng.standard_normal((B, L, IN_DIM), dtype=np.float32),
        "in_w": 0.02 * rng.standard_normal((D, IN_DIM), dtype=np.float32),
        "in_b": np.zeros(D, np.float32),
        "in_proj_w": 0.02 * rng.standard_normal((NL, 2 * E, D), dtype=np.float32),
        "conv_w": 0.1 * rng.standard_normal((NL, E, K), dtype=np.float32),
        "conv_b": np.zeros((NL, E), np.float32),
        "xproj_w": 0.02 * rng.standard_normal((NL, R + 2 * N, E), dtype=np.float32),
        "dtproj_w": 0.1 * rng.standard_normal((NL, E, R), dtype=np.float32),
        "dtproj_b": 0.5 * rng.standard_normal((NL, E), dtype=np.float32),
        "A_log": np.log(np.broadcast_to(np.arange(1, N + 1, dtype=np.float32), (NL, E, N))).copy(),
        "D_ssm": np.ones((NL, E), np.float32),
        "outproj_w": 0.02 * rng.standard_normal((NL, D, E), dtype=np.float32),
        "norm_w": np.ones((NL, D), np.float32),
        "normf_w": np.ones(D, np.float32),
        "out_w": 0.02 * rng.standard_normal((OUT_DIM, D), dtype=np.float32),
        "out_b": np.zeros(OUT_DIM, np.float32),
    }
    print(kernel(**ins))

